# revision 4
# baseline (speedup 1.0000x reference)
"""RBF-kernel dense layer (CustomKernelDense) on 8 Trainium2 NeuronCores.

out[b, u] = exp(-(||x_b||^2 + ||k_u||^2 - 2 x_b.k_u)) + bias[u]

Sharding: data-parallel over the batch dim. Core c computes rows
c*1024:(c+1)*1024 of the (8192, 4096) output; kernel/bias replicated.
No collectives -- the host concatenates the 8 output shards.

Device math per core (B_c=1024, D=512, U=4096):
  psum m[b,u]  = sum_d xT[d,b] * kern[d,u]        (4 K-chunks of 128, bf16)
  t[b,u]       = m + bc[u]     where bc[u] = -0.5*||k_u||^2   (DVE add)
  out[b,u]     = Exp(2*t + (-||x_b||^2))          (ACT, bias port = per-row)
             = exp(2*m - ||k_u||^2 - ||x_b||^2) = exp(-d2)
bias is added on the host after the gather (it is a (U,) vector applied
post-exp; for this problem it is identically zero).

The -0.5*||k_u||^2 broadcast rows are built on device: square the kernel
chunks (DVE), then matmul with a constant -0.5 [128,128] stationary operand,
which both reduces over d and replicates across all 128 partitions.
||x_b||^2 comes from ACT Square with accum_out on natural-layout x tiles.

bf16 operands: the moving-operand matmul runs at 1 cycle/row (fp32 is 4);
accumulation stays fp32 in PSUM. For these inputs d2 ~ 510 so exp
underflows to 0.0 in fp32 regardless of input rounding.
"""

import numpy as np
import ml_dtypes
from contextlib import ExitStack

B, D, U = 8192, 512, 4096
NCORES = 8
BC = B // NCORES  # 1024 batch rows per core
P = 128           # SBUF/PSUM partitions
KC = D // P       # 4 contraction chunks
NB = 512          # u-block width == one fp32 PSUM bank
UB = U // NB      # 8 u blocks
BT = BC // P      # 8 b tiles

_NC_CACHE = {}


def _build_nc(reps=1):
    import concourse.bass as bass
    import concourse.mybir as mybir
    import concourse.tile as tile
    from concourse import bacc

    dt = mybir.dt
    AF = mybir.ActivationFunctionType

    nc = bacc.Bacc(
        "TRN2", target_bir_lowering=False, debug=False, num_devices=NCORES
    )

    xT = nc.dram_tensor("xT", [D, BC], dt.bfloat16, kind="ExternalInput")
    xn = nc.dram_tensor("xn", [BC, D], dt.bfloat16, kind="ExternalInput")
    kern = nc.dram_tensor("kern", [D, U], dt.bfloat16, kind="ExternalInput")
    out = nc.dram_tensor("out", [BC, U], dt.float32, kind="ExternalOutput")

    def _body(tc, ctx):
        consts = ctx.enter_context(tc.tile_pool(name="consts", bufs=1))
        xnpool = ctx.enter_context(tc.tile_pool(name="xn", bufs=2))
        sqxpool = ctx.enter_context(tc.tile_pool(name="sqx", bufs=2))
        xsqpool = ctx.enter_context(tc.tile_pool(name="xsq", bufs=BT))
        negpool = ctx.enter_context(tc.tile_pool(name="negxsq", bufs=BT))
        kpool = ctx.enter_context(tc.tile_pool(name="kchunk", bufs=KC))
        xTpool = ctx.enter_context(tc.tile_pool(name="xTchunk", bufs=KC))
        sqkpool = ctx.enter_context(tc.tile_pool(name="sqk", bufs=KC))
        bcpool = ctx.enter_context(tc.tile_pool(name="bc", bufs=UB))
        tpool = ctx.enter_context(tc.tile_pool(name="t", bufs=4))
        opool = ctx.enter_context(tc.tile_pool(name="o", bufs=4))
        psum_m = ctx.enter_context(
            tc.tile_pool(name="psum_m", bufs=5, space=bass.MemorySpace.PSUM)
        )
        psum_bc = ctx.enter_context(
            tc.tile_pool(name="psum_bc", bufs=2, space=bass.MemorySpace.PSUM)
        )

        neghalf = consts.tile([P, P], dt.bfloat16)
        nc.vector.memset(neghalf[:], -0.5)

        # ---- per-row -||x_b||^2 columns (ACT bias operands) ----
        negxsq = []
        for bt in range(BT):
            xtile = xnpool.tile([P, D], dt.bfloat16)
            nc.sync.dma_start(xtile[:], xn[bt * P : (bt + 1) * P, :])
            sq = sqxpool.tile([P, D], dt.bfloat16)
            xsq = xsqpool.tile([P, 1], dt.float32)
            nc.scalar.activation(sq[:], xtile[:], AF.Square, accum_out=xsq[:])
            neg = negpool.tile([P, 1], dt.float32)
            nc.vector.tensor_scalar_mul(neg[:], xsq[:], -1.0)
            negxsq.append(neg)

        # ---- load kernel + xT chunks (contraction dim on partitions) ----
        kt = []
        for i in range(KC):
            t = kpool.tile([P, U], dt.bfloat16)
            nc.sync.dma_start(t[:], kern[i * P : (i + 1) * P, :])
            kt.append(t)
        xt = []
        for i in range(KC):
            t = xTpool.tile([P, BC], dt.bfloat16)
            nc.sync.dma_start(t[:], xT[i * P : (i + 1) * P, :])
            xt.append(t)

        # ---- -0.5*||k_u||^2 broadcast tiles ----
        sqk = []
        for i in range(KC):
            s = sqkpool.tile([P, U], dt.bfloat16)
            nc.vector.tensor_mul(s[:], kt[i][:], kt[i][:])
            sqk.append(s)
        bc = []
        for ub in range(UB):
            pb = psum_bc.tile([P, NB], dt.float32)
            for i in range(KC):
                nc.tensor.matmul(
                    pb[:],
                    neghalf[:],
                    sqk[i][:, ub * NB : (ub + 1) * NB],
                    start=(i == 0),
                    stop=(i == KC - 1),
                )
            t = bcpool.tile([P, NB], dt.float32)
            nc.vector.tensor_copy(t[:], pb[:])
            bc.append(t)

        # ---- main loop: matmul -> +bc -> exp -> store ----
        for ub in range(UB):
            for bt in range(BT):
                pm = psum_m.tile([P, NB], dt.float32)
                for i in range(KC):
                    nc.tensor.matmul(
                        pm[:],
                        xt[i][:, bt * P : (bt + 1) * P],
                        kt[i][:, ub * NB : (ub + 1) * NB],
                        start=(i == 0),
                        stop=(i == KC - 1),
                    )
                tt = tpool.tile([P, NB], dt.float32)
                nc.vector.tensor_tensor(
                    tt[:], pm[:], bc[ub][:], op=mybir.AluOpType.add
                )
                oo = opool.tile([P, NB], dt.float32)
                nc.scalar.activation(
                    oo[:], tt[:], AF.Exp, bias=negxsq[bt][:], scale=2.0
                )
                nc.sync.dma_start(
                    out[bt * P : (bt + 1) * P, ub * NB : (ub + 1) * NB], oo[:]
                )

    with tile.TileContext(nc) as tc, ExitStack() as ctx:
        if reps == 1:
            _body(tc, ctx)
        else:
            # Benchmark variant: repeat the full body inside one NEFF so
            # per-rep HW time can be extracted from wall-clock deltas.
            with tc.For_i(0, reps, 1):
                _body(tc, ctx)

    nc.compile()
    return nc


def _get_nc(reps=1):
    if reps not in _NC_CACHE:
        _NC_CACHE[reps] = _build_nc(reps)
    return _NC_CACHE[reps]


def _make_in_maps(x, kernel):
    xbf = x.astype(ml_dtypes.bfloat16)
    kbf = np.ascontiguousarray(kernel.astype(ml_dtypes.bfloat16))
    in_maps = []
    for c in range(NCORES):
        xs = xbf[c * BC : (c + 1) * BC]
        in_maps.append(
            {
                "xT": np.ascontiguousarray(xs.T),
                "xn": np.ascontiguousarray(xs),
                "kern": kbf,
            }
        )
    return in_maps


def _run(x, kernel, bias, trace=False, reps=1, **spmd_kwargs):
    from concourse.bass_utils import run_bass_kernel_spmd

    nc = _get_nc(reps)
    in_maps = _make_in_maps(x, kernel)
    res = run_bass_kernel_spmd(
        nc, in_maps, list(range(NCORES)), trace=trace, **spmd_kwargs
    )
    out = np.concatenate(
        [res.results[c]["out"] for c in range(NCORES)], axis=0
    )
    out = out + np.asarray(bias, np.float32)[None, :]
    return out.astype(np.float32, copy=False), res


def _bench(x, kernel, bias, reps_lo=1, reps_hi=17, iters=3):
    """Estimate per-execution HW time: wall(reps_hi) - wall(reps_lo) over
    (reps_hi - reps_lo) repetitions of the body inside one NEFF. RPC and
    host<->device transfer costs cancel in the difference."""
    import time

    # warm both NEFFs (compile + first dispatch)
    _run(x, kernel, bias, reps=reps_lo)
    _run(x, kernel, bias, reps=reps_hi)
    lo, hi = [], []
    for _ in range(iters):
        t0 = time.time()
        _run(x, kernel, bias, reps=reps_lo)
        lo.append(time.time() - t0)
        t0 = time.time()
        _run(x, kernel, bias, reps=reps_hi)
        hi.append(time.time() - t0)
    per_rep = (min(hi) - min(lo)) / (reps_hi - reps_lo)
    return per_rep, lo, hi


def kernel(x, kernel, bias):
    x = np.asarray(x, np.float32)
    kernel = np.asarray(kernel, np.float32)
    bias = np.asarray(bias, np.float32)
    assert x.shape == (B, D) and kernel.shape == (D, U) and bias.shape == (U,)
    out, _ = _run(x, kernel, bias)
    return out


# revision 22
# speedup vs baseline: 2.9625x; 2.9625x over previous
"""RBF-kernel dense layer (CustomKernelDense) on 8 Trainium2 NeuronCores.

out[b, u] = exp(-(||x_b||^2 + ||k_u||^2 - 2 x_b.k_u)) + bias[u]

Sharding: data-parallel over the batch dim. Core c computes rows
c*1024:(c+1)*1024 of the (8192, 4096) output; kernel/bias replicated.
No collectives -- the host concatenates the 8 output shards.

Device math per core (B_c=1024, D=512, U=4096):
  psum m[b,u]  = sum_d xT[d,b] * kern[d,u]        (4 K-chunks of 128, bf16)
  t[b,u]       = m + bc[u]     where bc[u] = -0.5*||k_u||^2   (DVE add)
  out[b,u]     = Exp(2*t + (-||x_b||^2))          (ACT, bias port = per-row)
             = exp(2*m - ||k_u||^2 - ||x_b||^2) = exp(-d2)
bias is added on the host after the gather (it is a (U,) vector applied
post-exp; for this problem it is identically zero).

The -0.5*||k_u||^2 broadcast rows are built on device: square the kernel
chunks (DVE), then matmul with a constant -0.5 [128,128] stationary operand,
which both reduces over d and replicates across all 128 partitions.
||x_b||^2 comes from ACT Square with accum_out on natural-layout x tiles.

bf16 operands: the moving-operand matmul runs at 1 cycle/row (fp32 is 4);
accumulation stays fp32 in PSUM. For these inputs d2 ~ 510 so exp
underflows to 0.0 in fp32 regardless of input rounding; worst-case general
rel err of the bf16 path is ~1e-2 on exp(-d2).

Measured steady-state ~102 us/exec/core on trn2 (22 MB DMA, 131k PE
row-cycles; DMA- and PE-bound about equally at the ridge).
"""

import numpy as np
import ml_dtypes
from contextlib import ExitStack

B, D, U = 8192, 512, 4096
NCORES = 8
BC = B // NCORES  # 1024 batch rows per core
P = 128           # SBUF/PSUM partitions
KC = D // P       # 4 contraction chunks
NB = 512          # u-block width == one fp32 PSUM bank
UB = U // NB      # 8 u blocks
BT = BC // P      # 8 b tiles

_NC_CACHE = {}


def _build_nc(reps=1, variant="full"):
    import concourse.bass as bass
    import concourse.mybir as mybir
    import concourse.tile as tile
    from concourse import bacc

    dt = mybir.dt
    AF = mybir.ActivationFunctionType

    nc = bacc.Bacc(
        "TRN2", target_bir_lowering=False, debug=False, num_devices=NCORES
    )

    xT = nc.dram_tensor("xT", [D, BC], dt.bfloat16, kind="ExternalInput")
    xn = nc.dram_tensor("xn", [BC, D], dt.bfloat16, kind="ExternalInput")
    kern = nc.dram_tensor("kern", [D, U], dt.bfloat16, kind="ExternalInput")
    out = nc.dram_tensor("out", [BC, U], dt.float32, kind="ExternalOutput")

    def _body(tc, ctx):
        if variant != "full":
            _body_variant(nc, tc, ctx, variant, dt, AF, xT, xn, kern, out)
            return
        W = 2 * NB      # epilogue/store super-tile width (2 PSUM banks)
        NW = U // W     # 4 super-blocks
        consts = ctx.enter_context(tc.tile_pool(name="consts", bufs=1))
        xnpool = ctx.enter_context(tc.tile_pool(name="xn", bufs=2))
        sqxpool = ctx.enter_context(tc.tile_pool(name="sqx", bufs=2))
        xsqpool = ctx.enter_context(tc.tile_pool(name="xsq", bufs=BT))
        negpool = ctx.enter_context(tc.tile_pool(name="negxsq", bufs=BT))
        # 2*KC bufs: lets iteration r+1's loads overlap iteration r's tail
        # in the benchmark loop; harmless address-space cost single-shot.
        kpool = ctx.enter_context(tc.tile_pool(name="kchunk", bufs=2 * KC))
        xTpool = ctx.enter_context(tc.tile_pool(name="xTchunk", bufs=2 * KC))
        sqkpool = ctx.enter_context(tc.tile_pool(name="sqk", bufs=KC))
        bcpool = ctx.enter_context(tc.tile_pool(name="bc", bufs=NW))
        tpool = ctx.enter_context(tc.tile_pool(name="t", bufs=3))
        opool = ctx.enter_context(tc.tile_pool(name="o", bufs=3))
        psum_m = ctx.enter_context(
            tc.tile_pool(name="psum_m", bufs=3, space=bass.MemorySpace.PSUM)
        )
        psum_bc = ctx.enter_context(
            tc.tile_pool(name="psum_bc", bufs=1, space=bass.MemorySpace.PSUM)
        )

        neghalf = consts.tile([P, P], dt.bfloat16)
        nc.vector.memset(neghalf[:], -0.5)

        # ---- load kernel + xT chunks first (phase-0 critical path), then
        # xn; loads go on the sync HWDGE queues, stores on gpsimd SWDGE so
        # input loads never queue behind output stores.
        kt = []
        for i in range(KC):
            t = kpool.tile([P, U], dt.bfloat16)
            nc.sync.dma_start(t[:], kern[i * P : (i + 1) * P, :])
            kt.append(t)
        xt = []
        for i in range(KC):
            t = xTpool.tile([P, BC], dt.bfloat16)
            nc.sync.dma_start(t[:], xT[i * P : (i + 1) * P, :])
            xt.append(t)

        # ---- per-row -||x_b||^2 columns (ACT bias operands) ----
        negxsq = []
        for bt in range(BT):
            xtile = xnpool.tile([P, D], dt.bfloat16)
            nc.sync.dma_start(xtile[:], xn[bt * P : (bt + 1) * P, :])
            sq = sqxpool.tile([P, D], dt.bfloat16)
            xsq = xsqpool.tile([P, 1], dt.float32)
            nc.scalar.activation(sq[:], xtile[:], AF.Square, accum_out=xsq[:])
            neg = negpool.tile([P, 1], dt.float32)
            nc.vector.tensor_scalar_mul(neg[:], xsq[:], -1.0)
            negxsq.append(neg)

        # ---- -0.5*||k_u||^2 broadcast tiles, one [P, W] per super-block:
        # matmul with a constant -0.5 stationary operand both reduces k^2
        # over d and replicates the row across all 128 partitions.
        sqk = []
        for i in range(KC):
            s = sqkpool.tile([P, U], dt.bfloat16)
            nc.vector.tensor_mul(s[:], kt[i][:], kt[i][:])
            sqk.append(s)
        bc = []
        for w in range(NW):
            pb = psum_bc.tile([P, W], dt.float32)
            for j in range(W // NB):
                for i in range(KC):
                    u0 = w * W + j * NB
                    nc.tensor.matmul(
                        pb[:, j * NB : (j + 1) * NB],
                        neghalf[:],
                        sqk[i][:, u0 : u0 + NB],
                        start=(i == 0),
                        stop=(i == KC - 1),
                    )
            t = bcpool.tile([P, W], dt.float32)
            nc.vector.tensor_copy(t[:], pb[:])
            bc.append(t)

        # ---- main loop: matmul -> +bc (DVE) -> exp (ACT bias) -> store ----
        for w in range(NW):
            for bt in range(BT):
                pm = psum_m.tile([P, W], dt.float32)
                for j in range(W // NB):
                    u0 = w * W + j * NB
                    for i in range(KC):
                        nc.tensor.matmul(
                            pm[:, j * NB : (j + 1) * NB],
                            xt[i][:, bt * P : (bt + 1) * P],
                            kt[i][:, u0 : u0 + NB],
                            start=(i == 0),
                            stop=(i == KC - 1),
                        )
                tt = tpool.tile([P, W], dt.float32)
                nc.vector.tensor_tensor(
                    tt[:], pm[:], bc[w][:], op=mybir.AluOpType.add
                )
                oo = opool.tile([P, W], dt.float32)
                nc.scalar.activation(
                    oo[:], tt[:], AF.Exp, bias=negxsq[bt][:], scale=2.0
                )
                nc.gpsimd.dma_start(
                    out[bt * P : (bt + 1) * P, w * W : (w + 1) * W], oo[:]
                )

    with tile.TileContext(nc) as tc, ExitStack() as ctx:
        if reps == 1:
            _body(tc, ctx)
        else:
            # Benchmark variant: repeat the full body inside one NEFF so
            # per-rep HW time can be extracted from wall-clock deltas.
            with tc.For_i(0, reps, 1):
                _body(tc, ctx)

    nc.compile()
    return nc


def _body_variant(nc, tc, ctx, variant, dt, AF, xT, xn, kern, out):
    """Stripped bodies for bottleneck bisection (bench-only)."""
    import concourse.mybir as mybir
    import concourse.bass as bass

    if variant == "null":
        pool = ctx.enter_context(tc.tile_pool(name="nullp", bufs=2))
        t = pool.tile([P, 8], dt.float32)
        nc.vector.memset(t[:], 0.0)
        nc.sync.dma_start(out[0:P, 0:8], t[:])
        return

    if variant == "dma":
        kpool = ctx.enter_context(tc.tile_pool(name="kchunk", bufs=KC))
        xTpool = ctx.enter_context(tc.tile_pool(name="xTchunk", bufs=KC))
        xnpool = ctx.enter_context(tc.tile_pool(name="xn", bufs=2))
        opool = ctx.enter_context(tc.tile_pool(name="o", bufs=1))
        for i in range(KC):
            t = kpool.tile([P, U], dt.bfloat16)
            nc.sync.dma_start(t[:], kern[i * P : (i + 1) * P, :])
        for i in range(KC):
            t = xTpool.tile([P, BC], dt.bfloat16)
            nc.sync.dma_start(t[:], xT[i * P : (i + 1) * P, :])
        for bt in range(BT):
            t = xnpool.tile([P, D], dt.bfloat16)
            nc.sync.dma_start(t[:], xn[bt * P : (bt + 1) * P, :])
        oo = opool.tile([P, NB], dt.float32)
        nc.vector.memset(oo[:], 0.0)
        for ub in range(UB):
            for bt in range(BT):
                nc.sync.dma_start(
                    out[bt * P : (bt + 1) * P, ub * NB : (ub + 1) * NB], oo[:]
                )
        return

    if variant == "pe":
        kpool = ctx.enter_context(tc.tile_pool(name="kchunk", bufs=KC))
        xTpool = ctx.enter_context(tc.tile_pool(name="xTchunk", bufs=KC))
        psum_m = ctx.enter_context(
            tc.tile_pool(name="psum_m", bufs=5, space=bass.MemorySpace.PSUM)
        )
        kt, xt = [], []
        for i in range(KC):
            t = kpool.tile([P, U], dt.bfloat16)
            nc.sync.dma_start(t[:], kern[i * P : (i + 1) * P, :])
            kt.append(t)
        for i in range(KC):
            t = xTpool.tile([P, BC], dt.bfloat16)
            nc.sync.dma_start(t[:], xT[i * P : (i + 1) * P, :])
            xt.append(t)
        for ub in range(UB):
            for bt in range(BT):
                pm = psum_m.tile([P, NB], dt.float32)
                for i in range(KC):
                    nc.tensor.matmul(
                        pm[:],
                        xt[i][:, bt * P : (bt + 1) * P],
                        kt[i][:, ub * NB : (ub + 1) * NB],
                        start=(i == 0),
                        stop=(i == KC - 1),
                    )
        return

    if variant == "epi":
        bcpool = ctx.enter_context(tc.tile_pool(name="bc", bufs=1))
        negpool = ctx.enter_context(tc.tile_pool(name="negxsq", bufs=1))
        tpool = ctx.enter_context(tc.tile_pool(name="t", bufs=4))
        opool = ctx.enter_context(tc.tile_pool(name="o", bufs=4))
        psum_m = ctx.enter_context(
            tc.tile_pool(name="psum_m", bufs=1, space=bass.MemorySpace.PSUM)
        )
        bc = bcpool.tile([P, NB], dt.float32)
        nc.vector.memset(bc[:], -250.0)
        neg = negpool.tile([P, 1], dt.float32)
        nc.vector.memset(neg[:], -250.0)
        pm = psum_m.tile([P, NB], dt.float32)
        nc.vector.memset(pm[:], 0.0)
        for ub in range(UB):
            for bt in range(BT):
                tt = tpool.tile([P, NB], dt.float32)
                nc.vector.tensor_tensor(
                    tt[:], pm[:], bc[:], op=mybir.AluOpType.add
                )
                oo = opool.tile([P, NB], dt.float32)
                nc.scalar.activation(
                    oo[:], tt[:], AF.Exp, bias=neg[:], scale=2.0
                )
        return

    raise ValueError(variant)


def _get_nc(reps=1, variant="full"):
    key = (reps, variant)
    if key not in _NC_CACHE:
        _NC_CACHE[key] = _build_nc(reps, variant)
    return _NC_CACHE[key]


def _make_in_maps(x, kernel):
    xbf = x.astype(ml_dtypes.bfloat16)
    kbf = np.ascontiguousarray(kernel.astype(ml_dtypes.bfloat16))
    in_maps = []
    for c in range(NCORES):
        sl = slice(c * BC, (c + 1) * BC)
        in_maps.append(
            {
                "xT": np.ascontiguousarray(xbf[sl].T),
                "xn": np.ascontiguousarray(xbf[sl]),
                "kern": kbf,
            }
        )
    return in_maps


def _run(x, kernel, bias, trace=False, reps=1, **spmd_kwargs):
    from concourse.bass_utils import run_bass_kernel_spmd

    nc = _get_nc(reps)
    in_maps = _make_in_maps(x, kernel)
    res = run_bass_kernel_spmd(
        nc, in_maps, list(range(NCORES)), trace=trace, **spmd_kwargs
    )
    out = np.concatenate(
        [res.results[c]["out"] for c in range(NCORES)], axis=0
    )
    out = out + np.asarray(bias, np.float32)[None, :]
    return out.astype(np.float32, copy=False), res


def _bench(x, kernel, bias, reps_lo=1025, reps_hi=4097, iters=3):
    """Estimate per-execution HW time: wall(reps_hi) - wall(reps_lo) over
    (reps_hi - reps_lo) repetitions of the body inside one NEFF. RPC and
    host<->device transfer costs cancel in the difference."""
    import time

    # warm both NEFFs (compile + first dispatch)
    _run(x, kernel, bias, reps=reps_lo)
    _run(x, kernel, bias, reps=reps_hi)
    lo, hi = [], []
    for _ in range(iters):
        t0 = time.time()
        _run(x, kernel, bias, reps=reps_lo)
        lo.append(time.time() - t0)
        t0 = time.time()
        _run(x, kernel, bias, reps=reps_hi)
        hi.append(time.time() - t0)
    per_rep = (min(hi) - min(lo)) / (reps_hi - reps_lo)
    return per_rep, lo, hi


def kernel(x, kernel, bias):
    x = np.asarray(x, np.float32)
    kernel = np.asarray(kernel, np.float32)
    bias = np.asarray(bias, np.float32)
    assert x.shape == (B, D) and kernel.shape == (D, U) and bias.shape == (U,)
    out, _ = _run(x, kernel, bias)
    return out
